# revision 16
# baseline (speedup 1.0000x reference)
# Trainium2 Bass kernel for nn_CircuitModel (Oja-rule sequential scan).
#
# Math: the reference scans  W <- W + lr*(y x^T - y^2 * W),  y_t = sigmoid(W_t x_t).
# Factor W's evolution per output row i:
#   W_t[i,:] = a_t[i]*W0[i,:] + sum_{s<t} (a_t[i]/a_{s+1}[i]) * lr * y_s[i] * x_s
#   a_t[i]   = prod_{s<t} (1 - lr*y_s[i]^2),  ln(1-lr*y^2) ~ -lr*y^2 (err < 5e-7)
# so with U = X W0^T (T x N) and G = X X^T (T x T):
#   pre_t[i] = a_t[i] * ( U[t,i] + sum_{s<t} beta_s[i]*G[s,t] ),  beta_s = lr*y_s/a_{s+1}
# The sequence is solved by a causal fixed-point iteration on y (Jacobian is
# strictly lower triangular, entries ~lr*G ~ 1e-2; 2 iterations reach ~2e-4):
#   p = y^2 ; cx = -lr*(Lx p) ; ci = -lr*(Li p)      (cumsums via matmul)
#   A = exp(cx) ; R = exp(-ci) ; beta' = 2*y*R       (lr/2 lives in the G mask)
#   pre = A * (U + Gm^T beta') ; th' = tanh(pre/2) ; y = 0.5 + 0.5*th
# sigmoid/ln are avoided so every ACT function (exp/tanh) lives in the single
# "exp_and_others" LUT set -> one LoadActFuncSet, triggered early by a dummy.
# The device returns th; the host applies y = 0.5 + 0.5*th after the gather.
#
# Layout: packed [128, 512] tiles, partition p = h*64+t (h = i-half), free =
# i within half.  W arrives HOST-TRANSPOSED (WT = W^T) so the U-matmul rhs
# loads straight from DRAM -- no PE transposes, no PSUM->SBUF repack.  X
# arrives as XTP = [X^T | 0 | X^T] ([1024, 192]): zero-padded 128-wide lhsT
# slices give U directly in the packed M=128 layout, and a strided [2,64]
# slice gives the tiled Gram [128,128] in one 8-matmul chain.  All loads are
# split into ~790-ns pieces over the three DMA queues (SP/ACT/Pool) so the
# last dependency lands early; U accumulates per column-chunk in two PSUM
# groups so each chunk's fixed point starts as soon as its half of W is in.
# The fixed point runs as two 256-column chunks software-pipelined across
# ACT/DVE/Pool/PE.  Sharding: pure batch parallel, one batch element per core.
import sys

sys.path.insert(0, "/opt/trn_rl_repo")

import numpy as np

import concourse.bacc as bacc
import concourse.mybir as mybir
from concourse.bass_utils import run_bass_kernel_spmd
from concourse.tile import TileContext

F32 = mybir.dt.float32
F32R = mybir.dt.float32r
AF = mybir.ActivationFunctionType
OP = mybir.AluOpType

B, T, N = 8, 64, 1024
LR = 1.0 / N
NITERS = 2
NH = N // 2  # 512
NC2 = NH // 2  # 256 (iteration chunk width)


def _build(reps=1):
    nc = bacc.Bacc(trn_type="TRN2")
    WTd = nc.dram_tensor("WT", [N, N], F32R, kind="ExternalInput")  # W^T
    XTPd = nc.dram_tensor("XTP", [N, 192], F32R, kind="ExternalInput")  # XT|0|XT
    Cd = nc.dram_tensor("CONSTS", [128, 384], F32R, kind="ExternalInput")  # LX|LI|GM
    Yd = nc.dram_tensor("Y", [T, N], F32, kind="ExternalOutput")  # holds th!

    with TileContext(nc) as tc:
        with (
            tc.tile_pool(name="big", bufs=1) as big,
            tc.tile_pool(name="it", bufs=2) as it,
            tc.tile_pool(name="up", bufs=1, space="PSUM") as up,
            tc.tile_pool(name="gp", bufs=1, space="PSUM") as gp,
            tc.tile_pool(name="cp", bufs=1, space="PSUM") as cp,
            tc.tile_pool(name="pp", bufs=2, space="PSUM") as pp,
        ):
            for _rep in range(reps):
                # ---- early scratch + PE clock warmup + ACT table trigger ----
                zj_sb = big.tile([128, NH], F32, tag="zj")
                nc.vector.memset(zj_sb[:, :], 0.0)
                jp_ps = gp.tile([64, 64], F32, tag="junk")
                for _ in range(6):
                    nc.tensor.matmul(
                        jp_ps[:, :],
                        zj_sb[:, 0:64],
                        zj_sb[:, 0:64],
                        start=True,
                        stop=True,
                        skip_group_check=True,
                    )

                # ---- DMAs: ~790ns pieces over SP / ACT / Pool queues ----
                xtp_sb = big.tile([128, 8 * 192], F32R, tag="xtp")
                xtp3o = xtp_sb[:, :].rearrange("p (kk c) -> p kk c", c=192)
                xtp3i = XTPd.rearrange("(kk p) c -> p kk c", p=128)
                nc.sync.dma_start(out=xtp3o[:, 0:3, :], in_=xtp3i[:, 0:3, :])
                nc.scalar.dma_start(out=xtp3o[:, 3:6, :], in_=xtp3i[:, 3:6, :])
                nc.gpsimd.dma_start(out=xtp3o[:, 6:8, :], in_=xtp3i[:, 6:8, :])
                # dummy activation: trigger the one LoadActFuncSet early
                za_sb = big.tile([128, 1], F32, tag="za")
                nc.scalar.activation(za_sb[:, :], zj_sb[:, 0:1], AF.Exp)

                # W^T pieces: (kk, jc) = contraction block kk, column-chunk jc.
                # Piece (kk,jc) holds cols {jc*256..} of BOTH i-halves.
                wt_sb = big.tile([128, 8 * N], F32R, tag="wt")
                wt5o = wt_sb[:, :].rearrange(
                    "p (kk jc h i) -> p kk jc h i", kk=8, jc=2, h=2, i=NC2
                )
                wt5i = WTd.rearrange(
                    "(kk p) (h jc i) -> p kk jc h i", p=128, h=2, jc=2, i=NC2
                )
                const_sb = big.tile([128, 384], F32R, tag="consts")
                # queue schedules (chunk-0 pieces first, consts mid-Pool)
                SP, ACT, PL = nc.sync, nc.scalar, nc.gpsimd
                sched = [
                    (SP, 0, 0), (PL, 1, 0), (ACT, 6, 0), (PL, 3, 0),
                    (SP, 2, 0), (PL, 5, 0), (ACT, 7, 0), (SP, 4, 0),
                    (PL, None, None),  # consts
                    (PL, 3, 1), (SP, 0, 1), (ACT, 6, 1), (SP, 1, 1),
                    (ACT, 7, 1), (PL, 4, 1), (SP, 2, 1), (PL, 5, 1),
                ]
                for q, kk, jc in sched:
                    if kk is None:
                        q.dma_start(out=const_sb[:, :], in_=Cd[:, :])
                    else:
                        q.dma_start(
                            out=wt5o[:, kk, jc, :, :], in_=wt5i[:, kk, jc, :, :]
                        )
                lx_sb = const_sb[:, 0:128]
                li_sb = const_sb[:, 128:256]
                gm_sb = const_sb[:, 256:384]

                # ---- tiled Gram [[G,G],[G,G]] via strided dup lhsT/rhs ----
                # contiguous [X^T X^T] dup (rhs needs one free dim on hw):
                # one strided Pool copy from the [X^T|0|X^T] layout
                xtp4 = xtp_sb[:, :].rearrange("p (kk d t) -> p kk d t", d=3, t=T)
                xtg_sb = big.tile([128, 8 * 128], F32R, tag="xtg")
                xtg4 = xtg_sb[:, :].rearrange("p (kk d t) -> p kk d t", d=2, t=T)
                nc.gpsimd.tensor_copy(xtg4[:, :, :, :], xtp4[:, :, 0:3:2, :])
                g_ps = gp.tile([128, 128], F32, tag="g")
                for kk in range(8):
                    xg = xtg_sb[:, kk * 128 : (kk + 1) * 128]
                    nc.tensor.matmul(
                        g_ps[:, :], xg, xg, start=(kk == 0), stop=(kk == 7)
                    )
                # mask to block-diag strict-upper, scaled by lr/2
                gmm_sb = big.tile([128, 128], F32R, tag="gmm")
                nc.vector.scalar_tensor_tensor(
                    gmm_sb[:, :], g_ps[:, :], 1.0, gm_sb[:, :], OP.mult, OP.mult
                )

                # ---- U = X W0^T, packed [h*64+t, i'], one PSUM group/chunk ----
                u0_ps = up.tile([128, NC2], F32, tag="u0")
                u1_ps = up.tile([128, NC2], F32, tag="u1")
                u_ps = [u0_ps, u1_ps]
                ndone = [0, 0]
                for q, kk, jc in sched:
                    if kk is None:
                        continue
                    for h in range(2):
                        lhsT = xtp_sb[:, kk * 192 + h * 64 : kk * 192 + h * 64 + 128]
                        nc.tensor.matmul(
                            u_ps[jc][:, :],
                            lhsT,
                            wt5o[:, kk, jc, h, :],
                            start=(ndone[jc] == 0),
                            stop=(ndone[jc] == 15),
                        )
                        ndone[jc] += 1

                # ---- seed: th0 = tanh(U/2), per chunk ----
                th_sb = it.tile([128, NH], F32, tag="th0")
                CH = [slice(0, NC2), slice(NC2, NH)]
                for j, S in enumerate(CH):
                    nc.scalar.activation(th_sb[:, S], u_ps[j][:, :], AF.Tanh, scale=0.5)

                # ---- fixed-point iterations, 2-chunk software pipeline ----
                # (GPSIMD cannot touch PSUM on hw: Pool gets the SBUF-only
                # ops p/beta, DVE the PSUM-facing preload and pre*A.  The
                # last iteration's dot accumulates straight into u_ps.)
                for k in range(NITERS):
                    last = k == NITERS - 1
                    if not last:
                        pre_ps = pp.tile([128, NH], F32, tag="pre")
                        pre = [pre_ps[:, CH[0]], pre_ps[:, CH[1]]]
                        for j in range(2):  # DVE: PSUM preload (off chain)
                            nc.vector.tensor_copy(pre[j], u_ps[j][:, :])
                    else:
                        pre = [u_ps[0][:, :], u_ps[1][:, :]]
                    y_sb = it.tile([128, NH], F32, tag="y")
                    p_sb = it.tile([128, NH], F32R, tag="p")
                    ci_ps = cp.tile([128, NH], F32, tag="ci")
                    cx_ps = cp.tile([128, NH], F32, tag="cx")
                    r_sb = it.tile([128, NH], F32, tag="r")
                    a_sb = it.tile([128, NH], F32, tag="a")
                    beta_sb = it.tile([128, NH], F32R, tag="beta")
                    pre_sb = it.tile([128, NH], F32, tag="presb")
                    nth_sb = it.tile([128, NH], F32, tag="th")
                    for S in CH:  # DVE: y2 = 1 + th  (= 2*y)
                        nc.vector.tensor_scalar_add(y_sb[:, S], th_sb[:, S], 1.0)
                    for S in CH:  # Pool: p' = y2*y2 = 4*y^2 (consts carry /4)
                        nc.gpsimd.tensor_tensor(
                            p_sb[:, S], y_sb[:, S], y_sb[:, S], OP.mult
                        )
                    for S in CH:  # PE: causal cumsums
                        nc.tensor.matmul(
                            ci_ps[:, S], li_sb, p_sb[:, S], start=True, stop=True
                        )
                        nc.tensor.matmul(
                            cx_ps[:, S], lx_sb, p_sb[:, S], start=True, stop=True
                        )
                    for S in CH:  # ACT: R = exp(-ci), A = exp(cx)
                        nc.scalar.activation(r_sb[:, S], ci_ps[:, S], AF.Exp, scale=-1.0)
                        nc.scalar.activation(a_sb[:, S], cx_ps[:, S], AF.Exp)
                    for S in CH:  # Pool: beta' = y2*R = 2*y*R
                        nc.gpsimd.tensor_tensor(
                            beta_sb[:, S], y_sb[:, S], r_sb[:, S], OP.mult
                        )
                    for j, S in enumerate(CH):  # PE: pre += Gm^T beta'
                        nc.tensor.matmul(
                            pre[j],
                            gmm_sb[:, :],
                            beta_sb[:, S],
                            start=False,
                            stop=True,
                            skip_group_check=True,
                        )
                    for j, S in enumerate(CH):  # DVE: pre_sb = pre * A
                        nc.vector.scalar_tensor_tensor(
                            pre_sb[:, S], pre[j], 1.0, a_sb[:, S], OP.mult, OP.mult
                        )
                    oq = [[SP, ACT], [PL, SP]]
                    for j, S in enumerate(CH):  # ACT: th' = tanh(pre/2)
                        nc.scalar.activation(
                            nth_sb[:, S], pre_sb[:, S], AF.Tanh, scale=0.5
                        )
                        if last:  # ship th per chunk as it completes
                            for h in range(2):
                                oq[j][h].dma_start(
                                    out=Yd[:, h * NH + j * NC2 : h * NH + (j + 1) * NC2],
                                    in_=nth_sb[h * T : (h + 1) * T, S],
                                )
                    th_sb = nth_sb
    nc.compile()
    return nc


_CACHE = {}


def _consts():
    lr = np.float64(LR)
    lx = np.triu(np.ones((T, T), np.float64), 1)  # lhsT[s,t]=1 iff s<t
    li = np.triu(np.ones((T, T), np.float64), 0)  # s<=t
    z = np.zeros((T, T), np.float64)
    # cumsum operand is p' = (1+th)^2 = 4*y^2 -> fold the 1/4 in here
    lxbd = (-0.25 * lr * np.block([[lx, z], [z, lx]])).astype(np.float32)
    libd = (-0.25 * lr * np.block([[li, z], [z, li]])).astype(np.float32)
    # strict-upper mask scaled by lr/2 (beta' = 2*y*R carries no lr)
    gmbd = (0.5 * lr * np.block([[lx, z], [z, lx]])).astype(np.float32)
    return np.ascontiguousarray(np.concatenate([lxbd, libd, gmbd], axis=1))


def _get_nc(reps=1):
    key = ("nc", reps)
    if key not in _CACHE:
        _CACHE[key] = _build(reps)
    return _CACHE[key]


def _in_maps(X, W_init):
    consts = _consts()
    maps = []
    for b in range(B):
        xt = np.ascontiguousarray(X[b].T, dtype=np.float32)  # [1024, 64]
        xtp = np.zeros((N, 192), dtype=np.float32)
        xtp[:, 0:T] = xt
        xtp[:, 128 : 128 + T] = xt
        maps.append(
            {
                "WT": np.ascontiguousarray(W_init[b].T, dtype=np.float32),
                "XTP": xtp,
                "CONSTS": consts,
            }
        )
    return maps


def kernel(X, W_init):
    nc = _get_nc()
    res = run_bass_kernel_spmd(nc, _in_maps(X, W_init), core_ids=list(range(B)))
    # device returns th = tanh(pre/2); y = sigmoid(pre) = 0.5 + 0.5*th
    Y = np.stack([res.results[b]["Y"] for b in range(B)], axis=0)
    return (0.5 + 0.5 * Y).astype(np.float32)
